# revision 13
# baseline (speedup 1.0000x reference)
"""CG transform (spherical-harmonic Clebsch-Gordan expansion) on 8 trn2 cores.

Math: for each triplet (l1,l2,l) and output row m, the reference computes
    H[m, f*F+g] = sum_{i,j} cg[m, i*k2+j] * f1[i,f] * f2[j,g]
which factorizes as two chained small matmuls:
    stage A:  WT[j, f]  = sum_i cg[m, i*k2+j] * f1[i, f]     (K=k1<=7, N=128)
    stage B:  out[f, g] = sum_j WT[j, f]     * f2[j, g]      (K=k2<=7, N=512x2)

Sharding: channel axis f of the FIRST kron factor is split across the 8
cores (core d owns f in [128d, 128d+128)).  Each core then produces the
full g-range for its f-slab of every output block: a contiguous
128*1024-column slice of each (2l+1, F^2) block.  No cross-core comms.

SBUF layouts: matmul operands must sit at 32-aligned partition bases, so
the 99 (triplet, m) pairs are assigned round-robin to the four 32-row
groups (base 32q, q = r % 4).  Host-side numpy builds the replicated /
block-aligned input layouts (pure data marshalling, no arithmetic).
"""

import sys

sys.path.insert(0, "/opt/trn_rl_repo")

import functools
from contextlib import ExitStack

import numpy as np

# ----------------------------------------------------------------------------
# Problem constants (hardcoded from the problem spec)
# ----------------------------------------------------------------------------
L = 3
F = 1024
NCORES = 8
FS = F // NCORES  # 128 channels per core

# Stage-B matmul operand dtype: "f32" (exact, 4 cyc/col) or "f32r"
# (1 cyc/col at N>=256, reduced-precision PE mode).
MM_DTYPE = "f32"


def _triplets(l_max):
    trips = []
    for l2 in range(l_max + 1):
        for l1 in range(l2 + 1):
            for l in range(l2 - l1, min(l_max, l1 + l2) + 1):
                trips.append((l1, l2, l))
    return trips


TRIPS = _triplets(L)  # 23 triplets, reference (cgc) order
# pairs: (triplet_index, m) in cgc-then-m order; r = index in this list
PAIRS = [(ti, m) for ti, (l1, l2, l) in enumerate(TRIPS) for m in range(2 * l + 1)]
NPAIRS = len(PAIRS)  # 99
NGROUPS = (NPAIRS + 3) // 4  # 25 column groups of <=4 pairs
R_OF = {pm: r for r, pm in enumerate(PAIRS)}

CGT_W = 8 * NPAIRS  # 792 -> pad to 800
CGT_WPAD = 800
INS_W = 512 + 4096 + CGT_WPAD  # f1t | cl2r | cgt packed into one input
WT_W = 128 * NGROUPS  # 3200
OUT_W = NPAIRS * F  # 101376


# ----------------------------------------------------------------------------
# Device program
# ----------------------------------------------------------------------------
@functools.lru_cache(maxsize=None)
def _build_program():
    import concourse.bacc as bacc
    import concourse.mybir as mybir
    import concourse.tile as tile

    f32 = mybir.dt.float32
    nc = bacc.Bacc(None)

    ins_d = nc.dram_tensor("ins", [128, INS_W], f32, kind="ExternalInput")
    out_d = nc.dram_tensor("out", [128, OUT_W], f32, kind="ExternalOutput")

    def mm_ap(ap):
        if MM_DTYPE == "f32r":
            return ap.bitcast(mybir.dt.float32r)
        return ap

    with tile.TileContext(nc) as tc, ExitStack() as ctx:
        const = ctx.enter_context(tc.tile_pool(name="const", bufs=1))
        psw = ctx.enter_context(tc.tile_pool(name="psw", bufs=2, space="PSUM"))
        psb = ctx.enter_context(tc.tile_pool(name="psb", bufs=4, space="PSUM"))
        sop = ctx.enter_context(tc.tile_pool(name="sop", bufs=3))

        ins = const.tile([128, INS_W], f32)
        wt = const.tile([128, WT_W], f32)

        nc.sync.dma_start(out=ins[:], in_=ins_d[:])
        f1t = ins[:, 0:512]
        cl2r = ins[:, 512 : 512 + 4096]
        cgt = ins[:, 4608 : 4608 + CGT_WPAD]

        ncopy = 0
        for c in range(NGROUPS):
            group = [(qi, 4 * c + qi) for qi in range(min(4, NPAIRS - 4 * c))]

            # ---- stage A: W^T[j, f] for each pair in this group ----
            pw = psw.tile([128, 128], f32, name=f"pw_{c}", tag="pw")
            for qi, r in group:
                ti, m = PAIRS[r]
                l1, l2, l = TRIPS[ti]
                k1, k2 = 2 * l1 + 1, 2 * l2 + 1
                q = 32 * qi
                nc.tensor.matmul(
                    pw[q : q + k2, 0:128],
                    lhsT=cgt[q : q + k1, 8 * r : 8 * r + k2],
                    rhs=f1t[q : q + k1, 128 * l1 : 128 * (l1 + 1)],
                    start=True,
                    stop=True,
                    tile_position=(q, q),
                )
            nc.vector.tensor_copy(wt[:, 128 * c : 128 * (c + 1)], pw[:, :])

            # ---- stage B: out[f, g] (two 512-col halves per pair) ----
            gsz = len(group)
            so = sop.tile([128, 4096], f32, name=f"so_{c}", tag="so")
            # Guard: absorb the WAR-vs-outbound-DMA wait on a cheap DVE op so
            # the real PSUM-evict copies carry <=2 sync waits (walrus limit).
            nc.vector.tensor_copy(so[:, 0:8], wt[:, 0:8])
            for qi, r in group:
                ti, m = PAIRS[r]
                l1, l2, l = TRIPS[ti]
                k2 = 2 * l2 + 1
                q = 32 * qi
                for h in range(2):
                    pb = psb.tile([128, 512], f32, name=f"pb_{c}_{qi}_{h}", tag="pb")
                    nc.tensor.matmul(
                        pb[:, :],
                        lhsT=mm_ap(wt[q : q + k2, 128 * c : 128 * (c + 1)]),
                        rhs=mm_ap(
                            cl2r[q : q + k2, 1024 * l2 + 512 * h : 1024 * l2 + 512 * (h + 1)]
                        ),
                        start=True,
                        stop=True,
                        tile_position=(q, 0),
                    )
                    dst = so[:, 1024 * qi + 512 * h : 1024 * qi + 512 * (h + 1)]
                    nc.vector.tensor_copy(dst, pb[:, :])
                    ncopy += 1

            nc.sync.dma_start(
                out=out_d[:, 4096 * c : 4096 * c + 1024 * gsz],
                in_=so[:, : 1024 * gsz],
            )

    nc.finalize()
    return nc


# ----------------------------------------------------------------------------
# Host-side input marshalling (pure layout, no arithmetic)
# ----------------------------------------------------------------------------
def _make_inputs(clms, cg_mats):
    clms = np.ascontiguousarray(np.asarray(clms, dtype=np.float32))
    assert clms.shape == ((L + 1) ** 2, F), clms.shape

    # cl2r: f2 block l2 replicated at every 32-row group base (for rhs of B)
    cl2r = np.zeros((128, 4096), np.float32)
    for q in range(4):
        for l2 in range(L + 1):
            k2 = 2 * l2 + 1
            cl2r[32 * q : 32 * q + k2, 1024 * l2 : 1024 * l2 + F] = clms[
                l2 * l2 : l2 * l2 + k2, :
            ]

    # cgt: per pair r, cg[t][m] reshaped (k1, k2) at partition base 32*(r%4)
    cgt = np.zeros((128, CGT_WPAD), np.float32)
    for r, (ti, m) in enumerate(PAIRS):
        l1, l2, l = TRIPS[ti]
        k1, k2 = 2 * l1 + 1, 2 * l2 + 1
        g = np.asarray(cg_mats[ti], dtype=np.float32)[m].reshape(k1, k2)
        cgt[32 * (r % 4) : 32 * (r % 4) + k1, 8 * r : 8 * r + k2] = g

    # f1t (per core): f1 block l1's slab columns at every 32-row group base
    in_maps = []
    for d in range(NCORES):
        f1t = np.zeros((128, 512), np.float32)
        for q in range(4):
            for l1 in range(L + 1):
                k1 = 2 * l1 + 1
                f1t[32 * q : 32 * q + k1, 128 * l1 : 128 * l1 + FS] = clms[
                    l1 * l1 : l1 * l1 + k1, FS * d : FS * (d + 1)
                ]
        ins = np.concatenate([f1t, cl2r, cgt], axis=1)
        in_maps.append({"ins": np.ascontiguousarray(ins)})
    return in_maps


def _assemble(core_outs):
    """core_outs: list of 8 arrays (128, OUT_W) -> tuple of 4 full outputs."""
    oa = np.stack([np.asarray(o) for o in core_outs])  # (8, 128, OUT_W)
    result = []
    for l in range(L + 1):
        tis = [ti for ti, (l1, l2, ll) in enumerate(TRIPS) if ll == l]
        n_l = len(tis)
        rows = 2 * l + 1
        h = np.empty((rows, n_l * F * F), np.float32)
        view = h.reshape(rows, n_l, NCORES, FS, F)
        for b, ti in enumerate(tis):
            for m in range(rows):
                r = R_OF[(ti, m)]
                view[m, b] = oa[:, :, F * r : F * (r + 1)]
        result.append(h)
    return tuple(result)


# ----------------------------------------------------------------------------
# Cached PJRT runner (compile once per process)
# ----------------------------------------------------------------------------
@functools.lru_cache(maxsize=None)
def _get_runner():
    import jax
    import jax.numpy as jnp
    from jax.sharding import Mesh, PartitionSpec
    from jax.experimental.shard_map import shard_map

    import concourse.mybir as mybir
    from concourse import bass2jax

    nc = _build_program()
    bass2jax.install_neuronx_cc_hook()
    assert nc.dbg_addr is None
    partition_name = (
        nc.partition_id_tensor.name if nc.partition_id_tensor else None
    )

    in_names, out_names, out_avals = [], [], []
    for alloc in nc.m.functions[0].allocations:
        if not isinstance(alloc, mybir.MemoryLocationSet):
            continue
        name = alloc.memorylocations[0].name
        if alloc.kind == "ExternalInput":
            if name != partition_name:
                in_names.append(name)
        elif alloc.kind == "ExternalOutput":
            out_names.append(name)
            out_avals.append(
                jax.core.ShapedArray(
                    tuple(alloc.tensor_shape), mybir.dt.np(alloc.dtype)
                )
            )
    n_params = len(in_names)
    n_outs = len(out_names)
    all_in_names = tuple(in_names) + tuple(out_names)
    if partition_name is not None:
        all_in_names = all_in_names + (partition_name,)

    def _body(*args):
        operands = list(args)
        if partition_name is not None:
            operands.append(bass2jax.partition_id_tensor())
        outs = bass2jax._bass_exec_p.bind(
            *operands,
            out_avals=tuple(out_avals),
            in_names=all_in_names,
            out_names=tuple(out_names),
            lowering_input_output_aliases=(),
            sim_require_finite=True,
            sim_require_nnan=True,
            nc=nc,
        )
        return tuple(outs)

    devices = jax.devices()[:NCORES]
    mesh = Mesh(np.asarray(devices), ("core",))
    in_specs = (PartitionSpec("core"),) * (n_params + n_outs)
    out_specs = (PartitionSpec("core"),) * n_outs
    donate = tuple(range(n_params, n_params + n_outs))
    sharded = jax.jit(
        shard_map(
            _body, mesh=mesh, in_specs=in_specs, out_specs=out_specs, check_rep=False
        ),
        donate_argnums=donate,
        keep_unused=True,
    )

    zero_shapes = [
        (NCORES * a.shape[0], *a.shape[1:]) for a in out_avals
    ]
    zero_dtypes = [a.dtype for a in out_avals]

    def run(in_maps, fetch=True, concat_in=None, zeros=None):
        if concat_in is None:
            concat_in = [
                np.concatenate([np.asarray(m[name]) for m in in_maps], axis=0)
                for name in in_names
            ]
        if zeros is None:
            zeros = [
                np.zeros(s, dt) for s, dt in zip(zero_shapes, zero_dtypes)
            ]
        out_arrs = sharded(*concat_in, *zeros)
        if not fetch:
            jax.block_until_ready(out_arrs)
            return None
        res = []
        for i in range(n_outs):
            full = np.asarray(out_arrs[i])
            per_core = full.reshape(NCORES, *out_avals[i].shape)
            res.append(per_core)
        return {name: res[i] for i, name in enumerate(out_names)}

    sharding = jax.sharding.NamedSharding(mesh, PartitionSpec("core"))

    def make_zeros():
        return [
            jax.jit(
                lambda s=s, dt=dt: jnp.zeros(s, dt), out_shardings=sharding
            )()
            for s, dt in zip(zero_shapes, zero_dtypes)
        ]

    run.in_names = in_names
    run.zero_shapes = zero_shapes
    run.zero_dtypes = zero_dtypes
    run.sharding = sharding
    run.make_zeros = make_zeros
    return run


# ----------------------------------------------------------------------------
# Public entry point
# ----------------------------------------------------------------------------
def kernel(clms, cg_mats):
    in_maps = _make_inputs(clms, cg_mats)
    run = _get_runner()
    outs = run(in_maps)["out"]  # (8, 128, OUT_W)
    return _assemble(list(outs))
